# revision 11
# baseline (speedup 1.0000x reference)
"""Trainium2 Bass kernel for multi-head self-attention with RoPE.

Problem: y = MHSA(x) with
    qkv = x @ W_qkv  -> (B,S,3,H,hd) -> per-head q,k,v
    q,k = rope(q), rope(k)   (interleaved-pair rotary)
    out  = softmax(q k^T / sqrt(hd)) v
    y    = concat_heads(out) @ W_out
B=2, S=2048, E=2048, H=16, hd=128.

Sharding: 8 cores; core c handles batch b=c//4 and 4 heads h0=4*(c%4)..h0+3
(tensor-parallel over heads, data-parallel over batch). Each core computes a
partial output  O_part = attn_heads @ W_out[rows of its heads]  and the host
sums the 4 partials per batch.

Schedule: one continuous PE stream with softmax exp (ACT engine, the serial
bottleneck of attention) hidden under matmul work:

  window 1: projection sweep for head-pair 0 (q/k+rope) + v for ALL heads
  window 2: projection sweep for head-pair 1 interleaved at matmul
            granularity with attention blocks for pair-0 heads
  window 3: attention blocks for pair-1 heads interleaved with the output
            projection; the last i-range of the output projection drains.

Attention blocks are software-pipelined: block u runs scores+exp for unit u
(a (head, i-range) pair) and attn@v for unit u-1, so the PE always has
independent work while ACT drains exp.

All device inputs are HOST-PREPACKED into the exact on-chip tile layouts so
every DMA line is multi-KB contiguous (the naive row-major layouts produced
254B descriptors and a ~30us DMA-bound ramp).

PSUM budget (8 banks): sc 2x[128,IHS]f32 (4) + ab 2x[128,512]f32 (2, v-proj
then attn@v accumulators) + cd 2x[128,512]f32 (2, warmup/rope-psum then
output-projection psum).
"""

import os
import math
import functools
from contextlib import ExitStack

import numpy as np

B, S, E = 2, 2048, 2048
A, H = 2048, 16
HD = A // H                     # 128
HPC = 4                         # heads per core
N_CORES = 8
THETA = 10000.0
SCALE = 1.0 / math.sqrt(HD)

LAST_RESULTS = None             # BassKernelResults of the last kernel() call


# --------------------------------------------------------------------------
# Bass program builder (parameterized so a small config can run in CoreSim)
# --------------------------------------------------------------------------
def build_bass(s=S, e=E, hpc=HPC, enable_asserts=False):
    import concourse.bass as bass
    import concourse.mybir as mybir
    import concourse.tile as tile
    from concourse import bacc

    f32 = mybir.dt.float32
    f16 = mybir.dt.float16
    Exp = mybir.ActivationFunctionType.Exp

    ES = e // 128               # contraction subtiles
    SC = 256                    # s-chunk width in projection sweeps
    NCH = s // SC               # chunks per sweep
    NJT = s // 128              # key tiles
    IHS = min(1024, s)          # i-range per attention unit
    IU = s // IHS               # i-units per head
    NPAIR = hpc // 2            # head pairs (sweeps)
    UPP = 2 * IU                # attention units per pair
    NU = hpc * IU               # total units
    VW = hpc * HD               # v width
    half = HD // 2

    nc = bacc.Bacc(
        "TRN2",
        target_bir_lowering=False,
        debug=False,
        enable_asserts=enable_asserts,
        num_devices=N_CORES,
    )

    xp = nc.dram_tensor("xp", (NCH, 128, ES, SC), f16, kind="ExternalInput").ap()
    wqk = nc.dram_tensor("wqk", (NPAIR, 4, 128, ES, 128), f16,
                         kind="ExternalInput").ap()
    wv = nc.dram_tensor("wv", (128, ES, VW), f16, kind="ExternalInput").ap()
    wop = nc.dram_tensor("wop", (128, hpc, e), f16, kind="ExternalInput").ap()
    cosP = nc.dram_tensor("cosP", (128, s), f16, kind="ExternalInput").ap()
    sinP = nc.dram_tensor("sinP", (128, s), f16, kind="ExternalInput").ap()
    O = nc.dram_tensor("O_part", (s, e), f16, kind="ExternalOutput").ap()

    # attention unit order: pair-0 units first (h-major), then pair-1 units
    # iu-major so each iu-group completes as early as possible for the
    # output projection.
    units = [(h, iu) for h in range(min(2, hpc)) for iu in range(IU)]
    for iu in range(IU):
        for h in (2, 3):
            if h < hpc:
                units.append((h, iu))

    with tile.TileContext(nc) as tc, ExitStack() as octx:
        # ---------------- persistent SBUF pools ---------------------------
        qkpool = octx.enter_context(tc.tile_pool(name="qkrot", bufs=1))
        qkrot = qkpool.tile([128, 2 * hpc, s], f16, tag="qkrot")
        vpool = octx.enter_context(tc.tile_pool(name="vsb", bufs=1))
        v_sb = vpool.tile([128, NJT, VW], f16, tag="vsb")
        o2pool = octx.enter_context(tc.tile_pool(name="o2pool", bufs=1))
        out2T = o2pool.tile([128, hpc, s], f16, tag="out2T")
        cpool = octx.enter_context(tc.tile_pool(name="cpool", bufs=1))
        ones16 = cpool.tile([128, 128], f16, tag="ones")
        tblpool = octx.enter_context(tc.tile_pool(name="tbl", bufs=1))
        cos_sb = tblpool.tile([128, s], f16, tag="cos")
        sin_sb = tblpool.tile([128, s], f16, tag="sin")
        accp = octx.enter_context(tc.tile_pool(name="accp", bufs=2))
        rbp = octx.enter_context(tc.tile_pool(name="rbp", bufs=1))
        rscp = octx.enter_context(tc.tile_pool(name="rscp", bufs=1))
        wqkpool = octx.enter_context(tc.tile_pool(name="wqkp", bufs=1))
        xpool = octx.enter_context(tc.tile_pool(name="xpool", bufs=2))
        wopool = octx.enter_context(tc.tile_pool(name="wopool", bufs=1))

        # ---------------- persistent PSUM pools (8 banks total) -----------
        p_sc = octx.enter_context(
            tc.tile_pool(name="p_sc", bufs=1, space="PSUM"))
        sc_big = p_sc.tile([128, 2, IHS], f32, tag="sc")
        p_ab = octx.enter_context(
            tc.tile_pool(name="p_ab", bufs=2, space="PSUM"))
        p_cd = octx.enter_context(
            tc.tile_pool(name="p_cd", bufs=2, space="PSUM"))

        xc_tiles = {}

        def load_xc(ch):
            t = xpool.tile([128, ES, SC], f16, tag="xc")
            nc.sync.dma_start(t[:], xp[ch])
            xc_tiles[ch] = t

        wqk_tiles = {}

        def load_wqk(p, t):
            w = wqkpool.tile([128, ES, 128], f16, tag=f"wqk{t}")
            nc.sync.dma_start(w[:], wqk[p, t])
            wqk_tiles[(p, t)] = w

        nc.vector.memset(ones16[:], 1.0)
        load_xc(0)
        for t in range(4):
            load_wqk(0, t)

        # ---------------- projection sweep (generator) --------------------
        RSCW = max(IHS, 4 * SC)

        def sweep(p, with_v, wv_sb, use_scbig):
            """Emits one head-pair's q/k projection + RoPE (and optionally
            the v projection for all heads), yielding after each matmul.
            use_scbig: rotate pse/pso through 4 quarters of sc_big (each in
            its own PSUM bank) for stall-free recycling; else use p_cd."""
            c0heads = 2 * p
            gctr = [0]

            nq = IHS // 512                  # 512-col bank slots per half
            nbuf = 2 * max(1, nq)

            def qk_psum(name):
                if use_scbig:
                    g = gctr[0] % nbuf
                    gctr[0] += 1
                    b, q = g // max(1, nq), g % max(1, nq)
                    return sc_big[:, b, q * 512:q * 512 + SC]
                return p_cd.tile([128, 512], f32, tag="cd", name=name)[:, 0:SC]

            def v_group(vch, stl):
                st = vch * (SC // 128) + stl
                vxc = xc_tiles[vch]
                ps = p_ab.tile([128, 512], f32, tag="ab", name="vps")[:, 0:VW]
                for es in range(ES):
                    nc.tensor.matmul(
                        ps[:], vxc[:, es, stl * 128:(stl + 1) * 128],
                        wv_sb[:, es, :],
                        start=(es == 0), stop=(es == ES - 1))
                    if es < ES - 1:
                        yield
                nc.vector.tensor_copy(v_sb[:, st, :], ps[:])
                yield

            pend = None
            for ch in range(NCH):
                if ch + 1 < NCH and (ch + 1) not in xc_tiles:
                    load_xc(ch + 1)
                cb = ch * SC
                cs = cos_sb[:, cb:cb + SC]
                sn = sin_sb[:, cb:cb + SC]
                xc = xc_tiles[ch]
                for qk in range(2):          # 0=q, 1=k
                    pse = qk_psum("pse")
                    we = wqk_tiles[(p, 2 * qk)]
                    for es in range(ES):
                        nc.tensor.matmul(
                            pse[:], we[:, es, :], xc[:, es, :],
                            start=(es == 0), stop=(es == ES - 1))
                        if es < ES - 1:
                            yield
                    yield
                    pso = qk_psum("pso")
                    wod = wqk_tiles[(p, 2 * qk + 1)]
                    for es in range(ES):
                        nc.tensor.matmul(
                            pso[:], wod[:, es, :], xc[:, es, :],
                            start=(es == 0), stop=(es == ES - 1))
                        if es < ES - 1:
                            yield
                    # RoPE: drain each PSUM buffer in its first two
                    # vector ops so the bank recycles fast; group-final
                    # yield so trailing ops are always emitted.
                    # quarters of rt: [pse*cs | pse*sn | pso*sn | pso*cs]
                    rt = rscp.tile([128, RSCW], f32, tag="rsc")
                    lo = qk * hpc + c0heads
                    hi = lo + 1
                    nc.vector.tensor_mul(rt[:, 0:SC], pse[:], cs)
                    nc.vector.tensor_mul(rt[:, SC:2 * SC], pse[:], sn)
                    nc.vector.tensor_mul(rt[:, 2 * SC:3 * SC], pso[:], sn)
                    nc.vector.tensor_mul(rt[:, 3 * SC:4 * SC], pso[:], cs)
                    nc.vector.tensor_sub(
                        qkrot[0:half, lo, cb:cb + SC],
                        rt[0:half, 0:SC], rt[0:half, 2 * SC:3 * SC])
                    nc.vector.tensor_sub(
                        qkrot[0:half, hi, cb:cb + SC],
                        rt[half:128, 0:SC], rt[half:128, 2 * SC:3 * SC])
                    nc.vector.tensor_add(
                        qkrot[half:HD, lo, cb:cb + SC],
                        rt[0:half, SC:2 * SC], rt[0:half, 3 * SC:4 * SC])
                    nc.vector.tensor_add(
                        qkrot[half:HD, hi, cb:cb + SC],
                        rt[half:128, SC:2 * SC], rt[half:128, 3 * SC:4 * SC])
                    yield
                    if with_v and pend is not None:
                        yield from v_group(pend, qk)
                        if qk == 1:
                            xc_tiles.pop(pend)
                if with_v:
                    pend = ch
                else:
                    xc_tiles.pop(ch)
            if with_v and pend is not None:
                for stl in range(SC // 128):
                    yield from v_group(pend, stl)
                xc_tiles.pop(pend)

        # ================= window 1: warmup + sweep 0 + all v =============
        with tc.tile_pool(name="wvp", bufs=1) as wvp:
            nc.sync.dma_start(cos_sb[:], cosP[:, :])
            nc.sync.dma_start(sin_sb[:], sinP[:, :])
            wv_sb = wvp.tile([128, ES, VW], f16, tag="wv")
            nc.sync.dma_start(wv_sb[:], wv)
            load_xc(1)

            # PE warmup on the ones tile: keeps HAM busy across the DMA
            # ramp; also primes the ACT exp table + DVE reciprocal ucode.
            wups = p_cd.tile([128, 512], f32, tag="cd")
            NWU = 56 if s >= 2048 else 6
            for i in range(NWU):
                nc.tensor.matmul(wups[:, 0:128], ones16[:], ones16[:],
                                 start=(i == 0), stop=(i == NWU - 1))
            prm = rscp.tile([128, RSCW], f32, tag="rsc")
            nc.scalar.activation(prm[:, 0:2], wups[:, 0:2], Exp, scale=SCALE)
            prm2 = rbp.tile([128, IHS], f32, tag="rb")
            nc.vector.reciprocal_approx_accurate(
                prm2[:, 0:2], wups[:, 2:4], prm[:, 2:4])

            for _ in sweep(0, True, wv_sb, use_scbig=True):
                pass

        # prefetch pair-1 weights + restart the x stream for sweep 1
        if NPAIR > 1:
            for t in range(4):
                load_wqk(1, t)
            load_xc(0)
            load_xc(1)

        # =========== windows 2+3: attention block pipeline ================
        NCK = min(512, IHS)
        NIC = IHS // NCK
        ECW = min(512, e)
        NEC = e // ECW

        exp_tiles = {}
        acc_tiles = {}
        up_live = {}

        with (
            tc.tile_pool(name="expp", bufs=2) as expp,
            tc.tile_pool(name="osb", bufs=2) as osbp,
        ):
            wo_sb = wopool.tile([128, hpc, e], f16, tag="wo")
            nc.sync.dma_start(wo_sb[:], wop)

            def emit_sc(u, jt):
                h, iu = units[u]
                i0 = iu * IHS
                if jt == 0:
                    exp_tiles[u] = expp.tile([128, NJT, IHS], f16, tag="expt",
                                             name="expt")
                ps = sc_big[:, jt % 2, :]
                for ic in range(NIC):
                    nc.tensor.matmul(
                        ps[:, ic * NCK:(ic + 1) * NCK],
                        qkrot[:, hpc + h, jt * 128:(jt + 1) * 128],
                        qkrot[:, h, i0 + ic * NCK:i0 + (ic + 1) * NCK])
                if jt % 2 == 1:
                    # one 2*IHS-wide exp per jt pair amortizes the ACT
                    # per-instruction overhead (~480ns) across 2048 cols
                    nc.scalar.activation(
                        exp_tiles[u][:, jt - 1:jt + 1, :], sc_big[:, :, :],
                        Exp, scale=SCALE)
                    acc = acc_tiles.get(u)
                    if acc is None:
                        acc = acc_tiles[u] = accp.tile(
                            [128, IHS], f16, tag="acc", name="acc")
                        nc.vector.tensor_copy(acc[:], exp_tiles[u][:, 0, :])
                    else:
                        nc.vector.tensor_add(acc[:], acc[:],
                                             exp_tiles[u][:, jt - 1, :])
                    nc.vector.tensor_add(acc[:], acc[:],
                                         exp_tiles[u][:, jt, :])

            def emit_av(u, jt):
                h, iu = units[u]
                if jt == 0:
                    up_live[u] = [p_ab.tile([128, 512], f32, tag="ab", name="up")
                                  for _ in range(NIC)]
                expt = exp_tiles[u]
                for ic in range(NIC):
                    nc.tensor.matmul(
                        up_live[u][ic][:, 0:NCK],
                        v_sb[:, jt, h * HD:(h + 1) * HD],
                        expt[:, jt, ic * NCK:(ic + 1) * NCK],
                        start=(jt == 0), stop=(jt == NJT - 1))

            def emit_unit_tail(u):
                h, iu = units[u]
                i0 = iu * IHS
                rb = rbp.tile([128, IHS], f32, tag="rb")
                rsc = rscp.tile([128, RSCW], f32, tag="rsc")
                for ic in range(NIC):
                    dn = p_cd.tile([128, 512], f32, tag="cd",
                                   name="dn")[:, 0:NCK]
                    nc.tensor.matmul(dn[:], ones16[:],
                                     acc_tiles[u][:, ic * NCK:(ic + 1) * NCK])
                    nc.vector.reciprocal_approx_accurate(
                        rb[:, ic * NCK:(ic + 1) * NCK], dn[:],
                        rsc[:, ic * NCK:(ic + 1) * NCK])
                for ic in range(NIC):
                    nc.vector.tensor_mul(
                        out2T[:, h, i0 + ic * NCK:i0 + (ic + 1) * NCK],
                        up_live[u][ic][:, 0:NCK],
                        rb[:, ic * NCK:(ic + 1) * NCK])
                del up_live[u], exp_tiles[u], acc_tiles[u]

            orow_tiles = {}

            def o_group(it, ec, eng):
                op = p_cd.tile([128, 512], f32, tag="cd", name="ops")[:, 0:ECW]
                for h in range(hpc):
                    nc.tensor.matmul(
                        op[:], out2T[:, h, it * 128:(it + 1) * 128],
                        wo_sb[:, h, ec * ECW:(ec + 1) * ECW],
                        start=(h == 0), stop=(h == hpc - 1))
                if it not in orow_tiles:
                    orow_tiles[it] = osbp.tile([128, e], f16, tag="osb",
                                               name="orow")
                ob = orow_tiles[it]
                if eng == 0:
                    nc.vector.tensor_copy(ob[:, ec * ECW:(ec + 1) * ECW], op[:])
                else:
                    nc.scalar.copy(ob[:, ec * ECW:(ec + 1) * ECW], op[:])
                if ec == NEC - 1:
                    nc.sync.dma_start(O[it * 128:(it + 1) * 128, :], ob[:])
                    del orow_tiles[it]

            o_queue = []

            def o_pull(n):
                for _ in range(n):
                    if not o_queue:
                        return
                    o_group(*o_queue.pop(0))

            filler = sweep(1, False, None, use_scbig=False) \
                if NPAIR > 1 else iter(())
            n_fill = NCH * 4 * ES if NPAIR > 1 else 0
            w2_slots = UPP * NJT
            fill_done = 0

            for bu in range(NU + 1):               # block index
                cur = bu if bu < NU else None          # unit doing scores
                prev = bu - 1 if bu >= 1 else None     # unit doing attn@v
                def pull_bg(jt):
                    nonlocal fill_done
                    if cur is not None and cur < UPP:
                        slot_idx = cur * NJT + jt
                        want = (n_fill * (slot_idx + 1)) // w2_slots
                        while fill_done < want:
                            try:
                                next(filler)
                                fill_done += 1
                            except StopIteration:
                                fill_done = n_fill
                    else:
                        o_pull(1)

                for jt in range(NJT):
                    if jt < 2:
                        pull_bg(jt)
                    if cur is not None:
                        emit_sc(cur, jt)
                    if prev is not None and jt >= 3:
                        emit_av(prev, jt - 3)
                    if jt >= 2:
                        pull_bg(jt)
                if prev is not None:
                    for jt in range(NJT - 3, NJT):
                        emit_av(prev, jt)
                    emit_unit_tail(prev)
                    # release output-projection work as iu-groups finish
                    _, iu = units[prev]
                    done_h = {units[v][0] for v in range(prev + 1)
                              if units[v][1] == iu}
                    if done_h == set(range(hpc)):
                        for it in range(iu * IHS // 128,
                                        (iu + 1) * IHS // 128):
                            for ec in range(NEC):
                                eng = len(o_queue) % 2
                                o_queue.append((it, ec, eng))
            o_pull(1 << 30)

    nc.compile()
    return nc


# --------------------------------------------------------------------------
# Host-side prep: sharding, packing into on-chip layouts, rope tables
# --------------------------------------------------------------------------
def host_prep(x, W_qkv, W_out, s=S, e=E, hpc=HPC, n_heads=H, n_batch=B,
              n_cores=N_CORES):
    a = n_heads * HD
    ES = e // 128
    SC = 256
    NCH = s // SC
    NPAIR = hpc // 2
    VW = hpc * HD

    inv = 1.0 / (THETA ** (np.arange(0, HD, 2, dtype=np.float64) / HD))
    fr = np.arange(s, dtype=np.float64)[:, None] * inv[None, :]
    cos = np.cos(fr).T
    sin = np.sin(fr).T
    cosP = np.ascontiguousarray(np.concatenate([cos, cos], axis=0)).astype(np.float16)
    sinP = np.ascontiguousarray(np.concatenate([sin, sin], axis=0)).astype(np.float16)

    cores_per_batch = n_cores // n_batch
    in_maps = []
    for c in range(n_cores):
        b = c // cores_per_batch
        h0 = hpc * (c % cores_per_batch)
        heads = [h0 + i for i in range(hpc)]

        # x packed chunk-major: [NCH, 128, ES, SC]
        xT = x[b].T.astype(np.float16)                       # (e, s)
        xpk = np.ascontiguousarray(
            xT.reshape(ES, 128, NCH, SC).transpose(2, 1, 0, 3))

        # qk weights: per pair p, tiles (q-even, q-odd, k-even, k-odd),
        # each tile = [evens|odds of head lo (64) , of head hi (64)]
        wqkp = np.empty((NPAIR, 4, 128, ES, 128), dtype=np.float16)
        for p in range(NPAIR):
            lo, hi = heads[2 * p], heads[2 * p + 1]
            for qk in range(2):
                off = qk * a
                for par in range(2):
                    cols = np.concatenate([
                        off + lo * HD + np.arange(par, HD, 2),
                        off + hi * HD + np.arange(par, HD, 2)])
                    wt = W_qkv[:, cols].astype(np.float16)   # (e, 128)
                    wqkp[p, 2 * qk + par] = \
                        wt.reshape(ES, 128, 128).transpose(1, 0, 2)
        wqkp = np.ascontiguousarray(wqkp)

        vcols = np.concatenate([2 * a + hh * HD + np.arange(HD)
                                for hh in heads])
        wvt = W_qkv[:, vcols].astype(np.float16)             # (e, VW)
        wvp = np.ascontiguousarray(wvt.reshape(ES, 128, VW).transpose(1, 0, 2))

        rows = np.concatenate([hh * HD + np.arange(HD) for hh in heads])
        wot = W_out[rows].astype(np.float16)                 # (VW, e)
        wop = np.ascontiguousarray(wot.reshape(hpc, 128, e).transpose(1, 0, 2))

        in_maps.append({
            "xp": xpk, "wqk": wqkp, "wv": wvp, "wop": wop,
            "cosP": cosP, "sinP": sinP,
        })
    return in_maps


@functools.lru_cache(maxsize=1)
def _get_nc():
    return build_bass()


def kernel(x, W_qkv, W_out):
    global LAST_RESULTS
    from concourse import bass_utils

    x = np.ascontiguousarray(np.asarray(x, dtype=np.float32))
    W_qkv = np.ascontiguousarray(np.asarray(W_qkv, dtype=np.float32))
    W_out = np.ascontiguousarray(np.asarray(W_out, dtype=np.float32))

    nc = _get_nc()
    in_maps = host_prep(x, W_qkv, W_out)
    trace = os.environ.get("KERNEL_TRACE", "0") == "1"
    res = bass_utils.run_bass_kernel_spmd(
        nc, in_maps, core_ids=list(range(N_CORES)), trace=trace,
    )
    LAST_RESULTS = res

    cores_per_batch = N_CORES // B
    O = np.zeros((B, S, E), dtype=np.float32)
    for c in range(N_CORES):
        O[c // cores_per_batch] += res.results[c]["O_part"]
    return O


# revision 14
# speedup vs baseline: 1.0460x; 1.0460x over previous
"""Trainium2 Bass kernel for multi-head self-attention with RoPE.

Problem: y = MHSA(x) with
    qkv = x @ W_qkv  -> (B,S,3,H,hd) -> per-head q,k,v
    q,k = rope(q), rope(k)   (interleaved-pair rotary)
    out  = softmax(q k^T / sqrt(hd)) v
    y    = concat_heads(out) @ W_out
B=2, S=2048, E=2048, H=16, hd=128.

Sharding: 8 cores; core c handles batch b=c//4 and 4 heads h0=4*(c%4)..h0+3
(tensor-parallel over heads, data-parallel over batch). Each core computes a
partial output  O_part = attn_heads @ W_out[rows of its heads]  and the host
sums the 4 partials per batch.

Schedule: one continuous PE stream with softmax exp (ACT engine, the serial
bottleneck of attention) hidden under matmul work:

  window 1: projection sweep for head-pair 0 (q/k+rope) + v for ALL heads
  window 2: projection sweep for head-pair 1 interleaved at matmul
            granularity with attention blocks for pair-0 heads
  window 3: attention blocks for pair-1 heads interleaved with the output
            projection; the last i-range of the output projection drains.

Attention blocks are software-pipelined: block u runs scores+exp for unit u
(a (head, i-range) pair) and attn@v for unit u-1, so the PE always has
independent work while ACT drains exp.

All device inputs are HOST-PREPACKED into the exact on-chip tile layouts so
every DMA line is multi-KB contiguous (the naive row-major layouts produced
254B descriptors and a ~30us DMA-bound ramp).

PSUM budget (8 banks): sc 2x[128,IHS]f32 (4) + ab 2x[128,512]f32 (2, v-proj
then attn@v accumulators) + cd 2x[128,512]f32 (2, warmup/rope-psum then
output-projection psum).
"""

import os
import math
import functools
from contextlib import ExitStack

import numpy as np

B, S, E = 2, 2048, 2048
A, H = 2048, 16
HD = A // H                     # 128
HPC = 4                         # heads per core
N_CORES = 8
THETA = 10000.0
SCALE = 1.0 / math.sqrt(HD)

LAST_RESULTS = None             # BassKernelResults of the last kernel() call


# --------------------------------------------------------------------------
# Bass program builder (parameterized so a small config can run in CoreSim)
# --------------------------------------------------------------------------
def build_bass(s=S, e=E, hpc=HPC, enable_asserts=False):
    import concourse.bass as bass
    import concourse.mybir as mybir
    import concourse.tile as tile
    from concourse import bacc

    f32 = mybir.dt.float32
    f16 = mybir.dt.float16
    Exp = mybir.ActivationFunctionType.Exp

    ES = e // 128               # contraction subtiles
    SC = 256                    # s-chunk width in projection sweeps
    NCH = s // SC               # chunks per sweep
    NJT = s // 128              # key tiles
    IHS = min(1024, s)          # i-range per attention unit
    IU = s // IHS               # i-units per head
    NPAIR = hpc // 2            # head pairs (sweeps)
    UPP = 2 * IU                # attention units per pair
    NU = hpc * IU               # total units
    VW = hpc * HD               # v width
    half = HD // 2

    nc = bacc.Bacc(
        "TRN2",
        target_bir_lowering=False,
        debug=False,
        enable_asserts=enable_asserts,
        num_devices=N_CORES,
    )

    xp = nc.dram_tensor("xp", (NCH, 128, ES, SC), f16, kind="ExternalInput").ap()
    wqk = nc.dram_tensor("wqk", (NPAIR, 4, 128, ES, 128), f16,
                         kind="ExternalInput").ap()
    wv = nc.dram_tensor("wv", (128, ES, VW), f16, kind="ExternalInput").ap()
    wop = nc.dram_tensor("wop", (128, hpc, e), f16, kind="ExternalInput").ap()
    cosP = nc.dram_tensor("cosP", (128, s), f16, kind="ExternalInput").ap()
    sinP = nc.dram_tensor("sinP", (128, s), f16, kind="ExternalInput").ap()
    O = nc.dram_tensor("O_part", (s, e), f16, kind="ExternalOutput").ap()

    # attention unit order: most pair-0 units first, then pair-1 iu-0
    # units, then pair-0's last unit (needs no new projections -> its block
    # absorbs the tail of the pair-1 q projection), then pair-1 iu-1.
    # This leaves no attention block without background matmul work except
    # the very first ones.
    units = [(h, iu) for h in range(min(2, hpc)) for iu in range(IU)]
    if hpc == 4:
        if IU > 1:
            lastp0 = units.pop()         # (1, IU-1)
            units += [(2, 0), (3, 0), lastp0]
            units += [(h, iu) for iu in range(1, IU) for h in (2, 3)]
        else:
            units += [(2, 0), (3, 0)]
    fp1 = next((i for i, (h, _) in enumerate(units) if h >= 2), len(units))
    fq2 = next((i for i, (h, iu) in enumerate(units)
                if h >= 2 and iu >= 1), len(units))

    with tile.TileContext(nc) as tc, ExitStack() as octx:
        # ---------------- persistent SBUF pools ---------------------------
        qkpool = octx.enter_context(tc.tile_pool(name="qkrot", bufs=1))
        qkrot = qkpool.tile([128, 2 * hpc, s], f16, tag="qkrot")
        vpool = octx.enter_context(tc.tile_pool(name="vsb", bufs=1))
        v_sb = vpool.tile([128, NJT, VW], f16, tag="vsb")
        o2pool = octx.enter_context(tc.tile_pool(name="o2pool", bufs=1))
        out2T = o2pool.tile([128, hpc, s], f16, tag="out2T")
        cpool = octx.enter_context(tc.tile_pool(name="cpool", bufs=1))
        ones16 = cpool.tile([128, 128], f16, tag="ones")
        tblpool = octx.enter_context(tc.tile_pool(name="tbl", bufs=1))
        cos_sb = tblpool.tile([128, s], f16, tag="cos")
        sin_sb = tblpool.tile([128, s], f16, tag="sin")
        accp = octx.enter_context(tc.tile_pool(name="accp", bufs=2))
        rbp = octx.enter_context(tc.tile_pool(name="rbp", bufs=1))
        rscp = octx.enter_context(tc.tile_pool(name="rscp", bufs=1))
        wqkpool = octx.enter_context(tc.tile_pool(name="wqkp", bufs=1))
        xpool = octx.enter_context(tc.tile_pool(name="xpool", bufs=2))
        wopool = octx.enter_context(tc.tile_pool(name="wopool", bufs=1))

        # ---------------- persistent PSUM pools (8 banks total) -----------
        p_sc = octx.enter_context(
            tc.tile_pool(name="p_sc", bufs=1, space="PSUM"))
        sc_big = p_sc.tile([128, 2, IHS], f32, tag="sc")
        p_ab = octx.enter_context(
            tc.tile_pool(name="p_ab", bufs=2, space="PSUM"))
        p_cd = octx.enter_context(
            tc.tile_pool(name="p_cd", bufs=2, space="PSUM"))

        xc_tiles = {}

        def load_xc(ch):
            t = xpool.tile([128, ES, SC], f16, tag="xc")
            nc.sync.dma_start(t[:], xp[ch])
            xc_tiles[ch] = t

        wqk_tiles = {}

        def load_wqk(p, t):
            w = wqkpool.tile([128, ES, 128], f16, tag=f"wqk{t}")
            nc.sync.dma_start(w[:], wqk[p, t])
            wqk_tiles[(p, t)] = w

        nc.vector.memset(ones16[:], 1.0)
        load_xc(0)
        for t in range(4):
            load_wqk(0, t)

        # ---------------- projection sweep (generator) --------------------
        RSCW = max(IHS, 4 * SC)

        def sweep(p, with_v, wv_sb, use_scbig):
            """Emits one head-pair's q/k projection + RoPE (and optionally
            the v projection for all heads), yielding after each matmul.
            use_scbig: rotate pse/pso through 4 quarters of sc_big (each in
            its own PSUM bank) for stall-free recycling; else use p_cd."""
            c0heads = 2 * p
            gctr = [0]

            nq = IHS // 512                  # 512-col bank slots per half
            nbuf = 2 * max(1, nq)

            def qk_psum(name):
                if use_scbig:
                    g = gctr[0] % nbuf
                    gctr[0] += 1
                    b, q = g // max(1, nq), g % max(1, nq)
                    return sc_big[:, b, q * 512:q * 512 + SC]
                return p_cd.tile([128, 512], f32, tag="cd", name=name)[:, 0:SC]

            def v_group(vch, stl):
                st = vch * (SC // 128) + stl
                vxc = xc_tiles[vch]
                ps = p_ab.tile([128, 512], f32, tag="ab", name="vps")[:, 0:VW]
                for es in range(ES):
                    nc.tensor.matmul(
                        ps[:], vxc[:, es, stl * 128:(stl + 1) * 128],
                        wv_sb[:, es, :],
                        start=(es == 0), stop=(es == ES - 1))
                    if es < ES - 1:
                        yield
                nc.vector.tensor_copy(v_sb[:, st, :], ps[:])
                yield

            pend = None
            for ch in range(NCH):
                if ch + 1 < NCH and (ch + 1) not in xc_tiles:
                    load_xc(ch + 1)
                cb = ch * SC
                cs = cos_sb[:, cb:cb + SC]
                sn = sin_sb[:, cb:cb + SC]
                xc = xc_tiles[ch]
                for qk in range(2):          # 0=q, 1=k
                    pse = qk_psum("pse")
                    we = wqk_tiles[(p, 2 * qk)]
                    for es in range(ES):
                        nc.tensor.matmul(
                            pse[:], we[:, es, :], xc[:, es, :],
                            start=(es == 0), stop=(es == ES - 1))
                        if es < ES - 1:
                            yield
                    yield
                    pso = qk_psum("pso")
                    wod = wqk_tiles[(p, 2 * qk + 1)]
                    for es in range(ES):
                        nc.tensor.matmul(
                            pso[:], wod[:, es, :], xc[:, es, :],
                            start=(es == 0), stop=(es == ES - 1))
                        if es < ES - 1:
                            yield
                    # RoPE: drain each PSUM buffer in its first two
                    # vector ops so the bank recycles fast; group-final
                    # yield so trailing ops are always emitted.
                    # quarters of rt: [pse*cs | pse*sn | pso*sn | pso*cs]
                    rt = rscp.tile([128, RSCW], f32, tag="rsc")
                    lo = qk * hpc + c0heads
                    hi = lo + 1
                    nc.vector.tensor_mul(rt[:, 0:SC], pse[:], cs)
                    nc.vector.tensor_mul(rt[:, SC:2 * SC], pse[:], sn)
                    nc.vector.tensor_mul(rt[:, 2 * SC:3 * SC], pso[:], sn)
                    nc.vector.tensor_mul(rt[:, 3 * SC:4 * SC], pso[:], cs)
                    nc.vector.tensor_sub(
                        qkrot[0:half, lo, cb:cb + SC],
                        rt[0:half, 0:SC], rt[0:half, 2 * SC:3 * SC])
                    nc.vector.tensor_sub(
                        qkrot[0:half, hi, cb:cb + SC],
                        rt[half:128, 0:SC], rt[half:128, 2 * SC:3 * SC])
                    nc.vector.tensor_add(
                        qkrot[half:HD, lo, cb:cb + SC],
                        rt[0:half, SC:2 * SC], rt[0:half, 3 * SC:4 * SC])
                    nc.vector.tensor_add(
                        qkrot[half:HD, hi, cb:cb + SC],
                        rt[half:128, SC:2 * SC], rt[half:128, 3 * SC:4 * SC])
                    yield
                    if with_v and pend is not None:
                        yield from v_group(pend, qk)
                        if qk == 1:
                            xc_tiles.pop(pend)
                if with_v:
                    pend = ch
                else:
                    xc_tiles.pop(ch)
            if with_v and pend is not None:
                for stl in range(SC // 128):
                    yield from v_group(pend, stl)
                xc_tiles.pop(pend)

        def sweep1_units(unit_list):
            """Window-2/3 filler: pair-1 projection units in the given
            (qk, ch) order (k units first, then q units so the late-i q
            chunks can fill the early pair-1 attention blocks).
            Yields after each matmul; prefetches the next unit's x chunk."""
            p = 1
            for i, (qk, ch) in enumerate(unit_list):
                if ch not in xc_tiles:
                    load_xc(ch)
                if i + 1 < len(unit_list) and unit_list[i + 1][1] not in xc_tiles:
                    load_xc(unit_list[i + 1][1])
                cb = ch * SC
                cs = cos_sb[:, cb:cb + SC]
                sn = sin_sb[:, cb:cb + SC]
                xc = xc_tiles[ch]
                pse = p_cd.tile([128, 512], f32, tag="cd", name="pse")[:, 0:SC]
                we = wqk_tiles[(p, 2 * qk)]
                for es in range(ES):
                    nc.tensor.matmul(
                        pse[:], we[:, es, :], xc[:, es, :],
                        start=(es == 0), stop=(es == ES - 1))
                    if es < ES - 1:
                        yield False
                yield False
                pso = p_cd.tile([128, 512], f32, tag="cd", name="pso")[:, 0:SC]
                wod = wqk_tiles[(p, 2 * qk + 1)]
                for es in range(ES):
                    nc.tensor.matmul(
                        pso[:], wod[:, es, :], xc[:, es, :],
                        start=(es == 0), stop=(es == ES - 1))
                    if es < ES - 1:
                        yield False
                rt = rscp.tile([128, RSCW], f32, tag="rsc")
                lo = qk * hpc + 2
                hi = lo + 1
                nc.vector.tensor_mul(rt[:, 0:SC], pse[:], cs)
                nc.vector.tensor_mul(rt[:, SC:2 * SC], pse[:], sn)
                nc.vector.tensor_mul(rt[:, 2 * SC:3 * SC], pso[:], sn)
                nc.vector.tensor_mul(rt[:, 3 * SC:4 * SC], pso[:], cs)
                nc.vector.tensor_sub(
                    qkrot[0:half, lo, cb:cb + SC],
                    rt[0:half, 0:SC], rt[0:half, 2 * SC:3 * SC])
                nc.vector.tensor_sub(
                    qkrot[0:half, hi, cb:cb + SC],
                    rt[half:128, 0:SC], rt[half:128, 2 * SC:3 * SC])
                nc.vector.tensor_add(
                    qkrot[half:HD, lo, cb:cb + SC],
                    rt[0:half, SC:2 * SC], rt[0:half, 3 * SC:4 * SC])
                nc.vector.tensor_add(
                    qkrot[half:HD, hi, cb:cb + SC],
                    rt[half:128, SC:2 * SC], rt[half:128, 3 * SC:4 * SC])
                yield True
                xc_tiles.pop(ch)

        # ================= window 1: warmup + sweep 0 + all v =============
        with tc.tile_pool(name="wvp", bufs=1) as wvp:
            nc.sync.dma_start(cos_sb[:], cosP[:, :])
            nc.sync.dma_start(sin_sb[:], sinP[:, :])
            wv_sb = wvp.tile([128, ES, VW], f16, tag="wv")
            nc.sync.dma_start(wv_sb[:], wv)
            load_xc(1)

            # PE warmup on the ones tile: keeps HAM busy across the DMA
            # ramp; also primes the ACT exp table + DVE reciprocal ucode.
            wups = p_cd.tile([128, 512], f32, tag="cd")
            NWU = 56 if s >= 2048 else 6
            for i in range(NWU):
                nc.tensor.matmul(wups[:, 0:128], ones16[:], ones16[:],
                                 start=(i == 0), stop=(i == NWU - 1))
            prm = rscp.tile([128, RSCW], f32, tag="rsc")
            nc.scalar.activation(prm[:, 0:2], wups[:, 0:2], Exp, scale=SCALE)
            prm2 = rbp.tile([128, IHS], f32, tag="rb")
            nc.vector.reciprocal_approx_accurate(
                prm2[:, 0:2], wups[:, 2:4], prm[:, 2:4])

            for _ in sweep(0, True, wv_sb, use_scbig=True):
                pass

        # prefetch pair-1 weights + restart the x stream for sweep 1
        if NPAIR > 1:
            for t in range(4):
                load_wqk(1, t)
            load_xc(0)
            load_xc(1)

        # =========== windows 2+3: attention block pipeline ================
        NCK = min(512, IHS)
        NIC = IHS // NCK
        ECW = min(512, e)
        NEC = e // ECW

        exp_tiles = {}
        acc_tiles = {}
        up_live = {}

        with (
            tc.tile_pool(name="expp", bufs=2) as expp,
            tc.tile_pool(name="osb", bufs=2) as osbp,
        ):
            wo_sb = wopool.tile([128, hpc, e], f16, tag="wo")
            nc.sync.dma_start(wo_sb[:], wop)

            def emit_sc(u, jt):
                h, iu = units[u]
                i0 = iu * IHS
                if jt == 0:
                    exp_tiles[u] = expp.tile([128, NJT, IHS], f16, tag="expt",
                                             name="expt")
                ps = sc_big[:, jt % 2, :]
                for ic in range(NIC):
                    nc.tensor.matmul(
                        ps[:, ic * NCK:(ic + 1) * NCK],
                        qkrot[:, hpc + h, jt * 128:(jt + 1) * 128],
                        qkrot[:, h, i0 + ic * NCK:i0 + (ic + 1) * NCK])
                if jt % 2 == 1:
                    # one 2*IHS-wide exp per jt pair amortizes the ACT
                    # per-instruction overhead (~480ns) across 2048 cols
                    nc.scalar.activation(
                        exp_tiles[u][:, jt - 1:jt + 1, :], sc_big[:, :, :],
                        Exp, scale=SCALE)
                    acc = acc_tiles.get(u)
                    if acc is None:
                        acc = acc_tiles[u] = accp.tile(
                            [128, IHS], f16, tag="acc", name="acc")
                        nc.vector.tensor_copy(acc[:], exp_tiles[u][:, 0, :])
                    else:
                        nc.vector.tensor_add(acc[:], acc[:],
                                             exp_tiles[u][:, jt - 1, :])
                    nc.vector.tensor_add(acc[:], acc[:],
                                         exp_tiles[u][:, jt, :])

            def emit_av(u, jt):
                h, iu = units[u]
                if jt == 0:
                    up_live[u] = [p_ab.tile([128, 512], f32, tag="ab", name="up")
                                  for _ in range(NIC)]
                expt = exp_tiles[u]
                for ic in range(NIC):
                    nc.tensor.matmul(
                        up_live[u][ic][:, 0:NCK],
                        v_sb[:, jt, h * HD:(h + 1) * HD],
                        expt[:, jt, ic * NCK:(ic + 1) * NCK],
                        start=(jt == 0), stop=(jt == NJT - 1))

            def emit_unit_tail(u):
                h, iu = units[u]
                i0 = iu * IHS
                rb = rbp.tile([128, IHS], f32, tag="rb")
                rsc = rscp.tile([128, RSCW], f32, tag="rsc")
                for ic in range(NIC):
                    dn = p_cd.tile([128, 512], f32, tag="cd",
                                   name="dn")[:, 0:NCK]
                    nc.tensor.matmul(dn[:], ones16[:],
                                     acc_tiles[u][:, ic * NCK:(ic + 1) * NCK])
                    nc.vector.reciprocal_approx_accurate(
                        rb[:, ic * NCK:(ic + 1) * NCK], dn[:],
                        rsc[:, ic * NCK:(ic + 1) * NCK])
                for ic in range(NIC):
                    nc.vector.tensor_mul(
                        out2T[:, h, i0 + ic * NCK:i0 + (ic + 1) * NCK],
                        up_live[u][ic][:, 0:NCK],
                        rb[:, ic * NCK:(ic + 1) * NCK])
                del up_live[u], exp_tiles[u], acc_tiles[u]

            orow_tiles = {}

            def o_group(it, ec, eng):
                op = p_cd.tile([128, 512], f32, tag="cd", name="ops")[:, 0:ECW]
                for h in range(hpc):
                    nc.tensor.matmul(
                        op[:], out2T[:, h, it * 128:(it + 1) * 128],
                        wo_sb[:, h, ec * ECW:(ec + 1) * ECW],
                        start=(h == 0), stop=(h == hpc - 1))
                if it not in orow_tiles:
                    orow_tiles[it] = osbp.tile([128, e], f16, tag="osb",
                                               name="orow")
                ob = orow_tiles[it]
                if eng == 0:
                    nc.vector.tensor_copy(ob[:, ec * ECW:(ec + 1) * ECW], op[:])
                else:
                    nc.scalar.copy(ob[:, ec * ECW:(ec + 1) * ECW], op[:])
                if it == s // 128 - 1:
                    nc.sync.dma_start(
                        O[it * 128:(it + 1) * 128, ec * ECW:(ec + 1) * ECW],
                        ob[:, ec * ECW:(ec + 1) * ECW])
                    if ec == NEC - 1:
                        del orow_tiles[it]
                elif ec == NEC - 1:
                    nc.sync.dma_start(O[it * 128:(it + 1) * 128, :], ob[:])
                    del orow_tiles[it]

            o_queue = []

            def o_pull(n):
                for _ in range(n):
                    if not o_queue:
                        return
                    o_group(*o_queue.pop(0))

            QL = (IHS // SC) if IU > 1 else NCH
            if NPAIR > 1:
                ulist = ([(1, ch) for ch in range(NCH)]
                         + [(0, ch) for ch in range(QL)]
                         + [(0, ch) for ch in range(QL, NCH)])
                filler = sweep1_units(ulist)
                n_a = (NCH + QL) * 2 * ES
                n_b = (NCH - QL) * 2 * ES
            else:
                filler = iter(())
                n_a = n_b = 0
            total_pulls_a = max(1, fp1) * (NJT + 2)
            total_pulls_b = max(1, fq2 - fp1) * (NJT + 2)
            fill_done = 0
            fill_safe = [True]
            pulls_a = 0
            pulls_b = 0

            for bu in range(NU + 1):               # block index
                cur = bu if bu < NU else None          # unit doing scores
                prev = bu - 1 if bu >= 1 else None     # unit doing attn@v
                def drain_to(want):
                    nonlocal fill_done
                    while fill_done < want:
                        try:
                            fill_safe[0] = next(filler)
                            fill_done += 1
                        except StopIteration:
                            fill_safe[0] = True
                            fill_done = want

                def flush_to_safe():
                    # the unit tail allocates PSUM from the filler's pool;
                    # emitting it mid-group would deadlock the vector queue
                    while not fill_safe[0]:
                        try:
                            fill_safe[0] = next(filler)
                        except StopIteration:
                            fill_safe[0] = True

                def pull_bg(jt):
                    nonlocal pulls_a, pulls_b
                    if cur is not None and cur < fp1:
                        pulls_a += 1
                        drain_to((n_a * pulls_a) // total_pulls_a)
                    elif (cur is not None and fp1 <= cur < fq2
                          and fill_done < n_a + n_b):
                        pulls_b += 1
                        drain_to(n_a + (n_b * pulls_b) // total_pulls_b)
                    else:
                        o_pull(1)

                for jt in range(NJT):
                    if jt < 2:
                        pull_bg(jt)
                    if cur is not None:
                        emit_sc(cur, jt)
                    if prev is not None and jt >= 3:
                        emit_av(prev, jt - 3)
                    if jt >= 2:
                        pull_bg(jt)
                if cur is not None:
                    pull_bg(NJT)
                    pull_bg(NJT + 1)
                flush_to_safe()
                if prev is not None:
                    for jt in range(NJT - 3, NJT):
                        emit_av(prev, jt)
                    emit_unit_tail(prev)
                    # release output-projection work as iu-groups finish
                    _, iu = units[prev]
                    done_h = {units[v][0] for v in range(prev + 1)
                              if units[v][1] == iu}
                    if done_h == set(range(hpc)):
                        for it in range(iu * IHS // 128,
                                        (iu + 1) * IHS // 128):
                            for ec in range(NEC):
                                eng = len(o_queue) % 2
                                o_queue.append((it, ec, eng))
            o_pull(1 << 30)

    nc.compile()
    return nc


# --------------------------------------------------------------------------
# Host-side prep: sharding, packing into on-chip layouts, rope tables
# --------------------------------------------------------------------------
def host_prep(x, W_qkv, W_out, s=S, e=E, hpc=HPC, n_heads=H, n_batch=B,
              n_cores=N_CORES):
    a = n_heads * HD
    ES = e // 128
    SC = 256
    NCH = s // SC
    NPAIR = hpc // 2
    VW = hpc * HD

    inv = 1.0 / (THETA ** (np.arange(0, HD, 2, dtype=np.float64) / HD))
    fr = np.arange(s, dtype=np.float64)[:, None] * inv[None, :]
    cos = np.cos(fr).T
    sin = np.sin(fr).T
    cosP = np.ascontiguousarray(np.concatenate([cos, cos], axis=0)).astype(np.float16)
    sinP = np.ascontiguousarray(np.concatenate([sin, sin], axis=0)).astype(np.float16)

    cores_per_batch = n_cores // n_batch
    in_maps = []
    for c in range(n_cores):
        b = c // cores_per_batch
        h0 = hpc * (c % cores_per_batch)
        heads = [h0 + i for i in range(hpc)]

        # x packed chunk-major: [NCH, 128, ES, SC]
        xT = x[b].T.astype(np.float16)                       # (e, s)
        xpk = np.ascontiguousarray(
            xT.reshape(ES, 128, NCH, SC).transpose(2, 1, 0, 3))

        # qk weights: per pair p, tiles (q-even, q-odd, k-even, k-odd),
        # each tile = [evens|odds of head lo (64) , of head hi (64)]
        wqkp = np.empty((NPAIR, 4, 128, ES, 128), dtype=np.float16)
        for p in range(NPAIR):
            lo, hi = heads[2 * p], heads[2 * p + 1]
            for qk in range(2):
                off = qk * a
                for par in range(2):
                    cols = np.concatenate([
                        off + lo * HD + np.arange(par, HD, 2),
                        off + hi * HD + np.arange(par, HD, 2)])
                    wt = W_qkv[:, cols].astype(np.float16)   # (e, 128)
                    wqkp[p, 2 * qk + par] = \
                        wt.reshape(ES, 128, 128).transpose(1, 0, 2)
        wqkp = np.ascontiguousarray(wqkp)

        vcols = np.concatenate([2 * a + hh * HD + np.arange(HD)
                                for hh in heads])
        wvt = W_qkv[:, vcols].astype(np.float16)             # (e, VW)
        wvp = np.ascontiguousarray(wvt.reshape(ES, 128, VW).transpose(1, 0, 2))

        rows = np.concatenate([hh * HD + np.arange(HD) for hh in heads])
        wot = W_out[rows].astype(np.float16)                 # (VW, e)
        wop = np.ascontiguousarray(wot.reshape(hpc, 128, e).transpose(1, 0, 2))

        in_maps.append({
            "xp": xpk, "wqk": wqkp, "wv": wvp, "wop": wop,
            "cosP": cosP, "sinP": sinP,
        })
    return in_maps


@functools.lru_cache(maxsize=1)
def _get_nc():
    return build_bass()


def kernel(x, W_qkv, W_out):
    global LAST_RESULTS
    from concourse import bass_utils

    x = np.ascontiguousarray(np.asarray(x, dtype=np.float32))
    W_qkv = np.ascontiguousarray(np.asarray(W_qkv, dtype=np.float32))
    W_out = np.ascontiguousarray(np.asarray(W_out, dtype=np.float32))

    nc = _get_nc()
    in_maps = host_prep(x, W_qkv, W_out)
    trace = os.environ.get("KERNEL_TRACE", "0") == "1"
    res = bass_utils.run_bass_kernel_spmd(
        nc, in_maps, core_ids=list(range(N_CORES)), trace=trace,
    )
    LAST_RESULTS = res

    cores_per_batch = N_CORES // B
    O = np.zeros((B, S, E), dtype=np.float32)
    for c in range(N_CORES):
        O[c // cores_per_batch] += res.results[c]["O_part"]
    return O
